# revision 2
# baseline (speedup 1.0000x reference)
"""Distributed k-NN retrieval kernel for Trainium2 (8 NeuronCores).

Problem: given query `key` [128], memory `keys` [1M, 128], `values` [1M, 128]:
  w_r = 1 / (||key - keys_r||^2 + 1e-3)            (all 1M rows)
  top-50 rows by w; output = sum_i (w_i / sum_all(w)) * values[i]   -> [1, 128]

Strategy: shard keys row-wise across 8 cores (125k rows each). Keys are
converted to bf16 on the host (halves HBM traffic; d error <= 0.6%,
validated) and sent transposed [128 feat, F rows]. Per core:
  - stream keysT bf16 in 31 tiles of [128, 4096]
  - squared differences, column-split across two engines:
      ScalarE: sq = Square(-k + q)   (fused subtract+square, q as bias)
      VectorE: diff = k - q; sq = diff*diff   (two bf16 passes)
  - TensorE (bf16, 1 cycle/col): ones[128,32] stationary at col-group
    tile_position (0, 32*pos); rhs = sq[:, 512-chunk] -> psum bank row
    range [32p, 32p+32) = d duplicated over 32 rows. 4 groups fill a
    [128, 512] PSUM bank.
  - VectorE: one [128, 512] psum->SBUF copy per bank into dall (bf16)
  - one strided DMA at the end ships rows {0,32,64,96} of each bank
    slice (the 4 distinct d vectors) to DRAM: d for all 126976 rows.
No on-device top-k: the host computes w = 1/(d+delta), the denominator,
an approximate top-512 pool from the bf16 d, exact fp32 rerank of the
pool (top-50 selection is then exact), and the weighted sum of values.
"""

import numpy as np

MAX_LEN = 1_000_000
N_KEY = 128
QUERY_WIDTH = 50
DELTA = np.float32(1e-3)
N_CORES = 8
ROWS_PER_CORE = 125_000        # 1M / 8 exactly
TILE = 4096                    # rows per DMA/compute tile
NTILES = 31
F = NTILES * TILE              # 126976 padded rows per core
GROUP = 512                    # rows per matmul (bass moving-operand max)
NGROUPS = F // GROUP           # 248
NBANKS = NGROUPS // 4          # 62 PSUM banks' worth
ACT_COLS = 2776                # columns of each tile squared on ScalarE
PAD_VAL = np.float32(1e9)      # pad rows -> d ~ 1.28e20 -> w ~ 0
POOL_K = 512                   # host rerank pool size

_NC_CACHE = {}


def _build_nc(rows=F, reps=1):
    """Build the per-core Bass program (identical on all cores).

    reps > 1 wraps the body in a device-side loop — used only for timing
    (marginal cost per rep isolates HW exec from dispatch overhead).
    """
    from contextlib import ExitStack, nullcontext

    import concourse.bacc as bacc
    import concourse.mybir as mybir
    import concourse.tile as tile

    f32 = mybir.dt.float32
    bf16 = mybir.dt.bfloat16

    assert rows % TILE == 0
    ntiles = rows // TILE
    nbanks = rows // (4 * GROUP)

    nc = bacc.Bacc(
        "TRN2",
        target_bir_lowering=False,
        debug=False,
        enable_asserts=False,
        num_devices=N_CORES,
    )
    keyst = nc.dram_tensor("keyst", [N_KEY, rows], bf16, kind="ExternalInput")
    qcol = nc.dram_tensor("qcol", [N_KEY, 1], f32, kind="ExternalInput")
    dout = nc.dram_tensor("dout", [rows // GROUP, GROUP], bf16,
                          kind="ExternalOutput")

    with tile.TileContext(nc) as tc, ExitStack() as ctx:
        constp = ctx.enter_context(tc.tile_pool(name="const", bufs=1))
        ktp = ctx.enter_context(tc.tile_pool(name="kt", bufs=4))
        sqp = ctx.enter_context(tc.tile_pool(name="sq", bufs=3))
        dfp = ctx.enter_context(tc.tile_pool(name="df", bufs=3))
        psp = ctx.enter_context(tc.tile_pool(name="ps", bufs=4, space="PSUM"))
        stp = ctx.enter_context(tc.tile_pool(name="stage", bufs=1))

        qs = constp.tile([N_KEY, 1], f32)
        nc.sync.dma_start(qs[:], qcol.ap())
        ones32 = constp.tile([N_KEY, 32], bf16)
        nc.vector.memset(ones32[:], 1.0)

        rep_ctx = tc.For_i(0, reps, 1) if reps > 1 else nullcontext()
        ctx.enter_context(rep_ctx)

        dall = stp.tile([128, nbanks * GROUP], bf16)
        ps = None

        for c in range(ntiles):
            kt = ktp.tile([N_KEY, TILE], bf16)
            nc.sync.dma_start(kt[:], keyst.ap()[:, c * TILE:(c + 1) * TILE])
            sq = sqp.tile([N_KEY, TILE], bf16)
            # ScalarE share: sq = Square(kt * -1 + q) = (q - k)^2
            nc.scalar.activation(
                sq[:, :ACT_COLS],
                kt[:, :ACT_COLS],
                mybir.ActivationFunctionType.Square,
                bias=qs[:],
                scale=-1.0,
            )
            # VectorE share: diff = k - q (bf16); sq = diff * diff
            df = dfp.tile([N_KEY, TILE - ACT_COLS], bf16)
            nc.vector.tensor_scalar(
                df[:], kt[:, ACT_COLS:], qs[:], None, mybir.AluOpType.subtract
            )
            nc.vector.tensor_tensor(
                sq[:, ACT_COLS:], df[:], df[:], mybir.AluOpType.mult
            )
            for j in range(TILE // GROUP):
                g = c * (TILE // GROUP) + j   # global 512-row group
                b, pos = g // 4, g % 4
                if pos == 0:
                    ps = psp.tile([128, GROUP], f32)
                # psum[32*pos + m, n] = d(row g*512 + n)  (dup over m)
                nc.tensor.matmul(
                    ps[32 * pos:32 * pos + 32, :],
                    ones32[:],
                    sq[:, j * GROUP:(j + 1) * GROUP],
                    start=True,
                    stop=True,
                    tile_position=(0, 32 * pos),
                )
                if pos == 3:
                    nc.vector.tensor_copy(
                        dall[:, b * GROUP:(b + 1) * GROUP], ps[:]
                    )
        # rows {0,32,64,96} of each bank slice hold the 4 distinct d
        # vectors; one strided DMA ships them as dout[4b + p4, :].
        src = dall[:].rearrange(
            "(a b) (c n) -> a b c n", a=4, c=nbanks
        )[:, 0]
        dst = dout.ap().rearrange("(c a) n -> a c n", a=4)
        nc.sync.dma_start(dst, src)

    nc.compile()
    return nc


def _get_nc(rows=F):
    if rows not in _NC_CACHE:
        _NC_CACHE[rows] = _build_nc(rows)
    return _NC_CACHE[rows]


def _make_shards(key, keys):
    """Host-side: bf16-convert + transpose + pad keys into [128, F] shards."""
    import jax
    import jax.numpy as jnp

    qcol = np.ascontiguousarray(
        np.asarray(key, dtype=np.float32).reshape(N_KEY, 1)
    )
    cpu = jax.devices("cpu")[0]
    with jax.default_device(cpu):
        kbT = np.asarray(
            jnp.transpose(jnp.asarray(keys).astype(jnp.bfloat16))
        )  # [128, 1M] ml_dtypes.bfloat16
    bf = kbT.dtype
    in_maps = []
    for c in range(N_CORES):
        base = c * ROWS_PER_CORE
        sh = np.full((N_KEY, F), PAD_VAL.astype(bf), dtype=bf)
        sh[:, :ROWS_PER_CORE] = kbT[:, base:base + ROWS_PER_CORE]
        in_maps.append({"keyst": sh, "qcol": qcol})
    return in_maps


def _merge(results, key, keys, values):
    """Host-side: d values -> exact top-50 -> weighted sum of values."""
    key = np.asarray(key, dtype=np.float32)
    d_parts = [
        np.asarray(r["dout"], dtype=np.float32).reshape(-1)[:ROWS_PER_CORE]
        for r in results
    ]
    dhat = np.concatenate(d_parts)           # [1M] approx d (bf16-rounded)
    w = np.float32(1.0) / (dhat + DELTA)
    W = np.float32(w.sum(dtype=np.float64))  # global denominator

    pool = np.argpartition(-w, POOL_K)[:POOL_K]
    dex = np.sum(
        (key[None, :] - keys[pool]) ** 2, axis=1, dtype=np.float32
    )
    wex = np.float32(1.0) / (dex + DELTA)
    # exact top-50 by weight; ties broken by lowest row index
    sel = np.lexsort((pool, -wex))[:QUERY_WIDTH]
    rows50 = pool[sel]
    weights = (wex[sel] / W).astype(np.float32)
    out = np.sum(
        values[rows50].astype(np.float32) * weights[:, None],
        axis=0,
        keepdims=True,
        dtype=np.float32,
    )
    return out.astype(np.float32)


_RUNNER_CACHE = {}


def _make_runner(nc, n_cores=N_CORES):
    """Reusable jitted PJRT executor for the SPMD program (axon path)."""
    import jax
    from jax.sharding import Mesh, NamedSharding, PartitionSpec

    try:
        from jax.experimental.shard_map import shard_map
    except ImportError:
        shard_map = jax.shard_map
    import concourse.bass2jax as b2j
    import concourse.mybir as mybir

    b2j.install_neuronx_cc_hook()

    partition_name = (
        nc.partition_id_tensor.name if nc.partition_id_tensor else None
    )
    in_names, out_names, out_avals, zero_outs = [], [], [], []
    for alloc in nc.m.functions[0].allocations:
        if not isinstance(alloc, mybir.MemoryLocationSet):
            continue
        if not alloc.memorylocations:
            continue
        name = alloc.memorylocations[0].name
        if alloc.kind == "ExternalInput":
            if name != partition_name:
                in_names.append(name)
        elif alloc.kind == "ExternalOutput":
            shape = tuple(alloc.tensor_shape)
            dtype = mybir.dt.np(alloc.dtype)
            out_names.append(name)
            out_avals.append(jax.core.ShapedArray(shape, dtype))
            zero_outs.append(np.zeros(shape, dtype))
    n_params = len(in_names)
    all_names = in_names + out_names
    if partition_name is not None:
        all_names.append(partition_name)
    donate = tuple(range(n_params, n_params + len(out_names)))

    def _body(*args):
        operands = list(args)
        if partition_name is not None:
            operands.append(b2j.partition_id_tensor())
        outs = b2j._bass_exec_p.bind(
            *operands,
            out_avals=tuple(out_avals),
            in_names=tuple(all_names),
            out_names=tuple(out_names),
            lowering_input_output_aliases=(),
            sim_require_finite=True,
            sim_require_nnan=True,
            nc=nc,
        )
        return tuple(outs)

    devices = jax.devices()[:n_cores]
    mesh = Mesh(np.asarray(devices), ("core",))
    fn = jax.jit(
        shard_map(
            _body,
            mesh=mesh,
            in_specs=(PartitionSpec("core"),) * (n_params + len(out_names)),
            out_specs=(PartitionSpec("core"),) * len(out_names),
            check_rep=False,
        ),
        donate_argnums=donate,
        keep_unused=True,
    )
    sh = NamedSharding(mesh, PartitionSpec("core"))

    def run(in_maps):
        cin = [
            jax.device_put(
                np.concatenate([m[name] for m in in_maps], axis=0), sh
            )
            for name in in_names
        ]
        zz = [
            jax.device_put(
                np.zeros((n_cores * z.shape[0], *z.shape[1:]), z.dtype), sh
            )
            for z in zero_outs
        ]
        out_arrs = fn(*cin, *zz)
        jax.block_until_ready(out_arrs)
        return [
            {
                name: np.asarray(out_arrs[i]).reshape(
                    n_cores, *out_avals[i].shape
                )[c]
                for i, name in enumerate(out_names)
            }
            for c in range(n_cores)
        ]

    return run


def kernel(key, keys, values, _collect_perf=None):
    """Full-input, full-output entry point. Shards across 8 NeuronCores."""
    nc = _get_nc()
    if F not in _RUNNER_CACHE:
        _RUNNER_CACHE[F] = _make_runner(nc)
    in_maps = _make_shards(np.asarray(key), np.asarray(keys))
    results = _RUNNER_CACHE[F](in_maps)
    if _collect_perf is not None:
        _collect_perf["results"] = results
    return _merge(results, np.asarray(key), np.asarray(keys),
                  np.asarray(values))
